# revision 1
# baseline (speedup 1.0000x reference)
"""Trainium2 Bass kernel for MultiHeadAttention (RMSNorm + MHA + residual).

Reference computation (B=2, S=2048, D=1024, H=16):
    xn = x * rsqrt(mean(x^2, -1) + 1e-12) * gamma
    q/k/v = (xn @ W{q,k,v}.T) split into heads
    attn  = softmax(q k^T / sqrt(64)) v          (mask is zeros)
    out   = xn + (attn @ Wo.T)

Sharding: tensor-parallel over heads (2 heads/core on 8 cores) for
QKV/scores/softmax/attn-V, then an AllToAll switches to token sharding
(512 tokens/core) for the output projection + residual, so each core
returns its own row-slice of the final output (no all-reduce needed).

Device layout: activations are feature-major ("T" = [feature, token]) so
every matmul contraction lands on the partition dim. x arrives from the
host in both token-major (for the mean-square reduce) and feature-major
bf16 copies, which avoids any on-device transpose of x. Softmax runs on
transposed scores [key, query]: one fused exp per key-tile covers both
heads, and the denominator Z falls out of the attn @ V matmul by
augmenting V with 64 ones-columns (Z lands replicated on psum
partitions 64..127). gamma is folded into Wq/Wk/Wv host-side.
"""

import numpy as np
import ml_dtypes

import concourse.bacc as bacc
import concourse.mybir as mybir
import concourse.tile as tile
from concourse.bass_utils import run_bass_kernel_spmd
from concourse.masks import make_identity

F32 = mybir.dt.float32
BF16 = mybir.dt.bfloat16
AF = mybir.ActivationFunctionType
ALU = mybir.AluOpType

NCORES = 8
D = 1024
H = 16
DH = 64            # head dim
HPC = H // NCORES  # heads per core
FPC = HPC * DH     # attn features per core


def build(B=2, S=2048, no_transpose=False, no_bcast=False,
          no_merged_exp=False, stop_after=None):
    TOK = B * S
    NT = TOK // 128      # token tiles
    IC = D // 128        # input-feature chunks
    TG = TOK // 512      # token groups for Q/K/V projections
    TPC = TOK // NCORES  # tokens per core (= one A2A shard / q-block)
    LT = TPC // 128      # local token tiles
    KT = S // 128        # key tiles per batch
    QCH = TPC            # q-block size (one A2A shard)
    QQ = S // QCH        # q-blocks per batch
    assert TPC % 128 == 0 and S % TPC == 0 and TPC <= 512

    nc = bacc.Bacc("TRN2", target_bir_lowering=False, debug=False,
                   num_devices=NCORES)
    xb_d = nc.dram_tensor("xb", [TOK, D], BF16, kind="ExternalInput")
    xt_d = nc.dram_tensor("xt", [D, TOK], BF16, kind="ExternalInput")
    xres_d = nc.dram_tensor("xres", [TPC, D], F32, kind="ExternalInput")
    wq_d = nc.dram_tensor("wq", [D, FPC], BF16, kind="ExternalInput")
    wk_d = nc.dram_tensor("wk", [D, FPC], BF16, kind="ExternalInput")
    wv_d = nc.dram_tensor("wv", [D, FPC], BF16, kind="ExternalInput")
    wo_d = nc.dram_tensor("wo", [D, D], BF16, kind="ExternalInput")
    gamma_d = nc.dram_tensor("gamma", [1, D], F32, kind="ExternalInput")
    out_d = nc.dram_tensor("out", [TPC, D], F32, kind="ExternalOutput")

    with tile.TileContext(nc) as tc:
        with (
            tc.tile_pool(name="sb", bufs=1) as sb,
            tc.tile_pool(name="ps", bufs=1, space="PSUM") as ps,
            tc.tile_pool(name="dram", bufs=1, space="DRAM") as dpool,
        ):
            bounce_in = dpool.tile([NCORES, FPC, TPC], BF16)
            bounce_out = dpool.tile([NCORES, FPC, TPC], BF16)
            rs_dram = dpool.tile([128, NT], BF16)

            # ---- persistent weights / constants ----
            wq_sb = sb.tile([128, IC, FPC], BF16, tag="wq")
            wk_sb = sb.tile([128, IC, FPC], BF16, tag="wk")
            wv_sb = sb.tile([128, IC, FPC], BF16, tag="wv")
            for w_sb, w_d in ((wq_sb, wq_d), (wk_sb, wk_d), (wv_sb, wv_d)):
                nc.sync.dma_start(
                    w_sb[:], w_d[:].rearrange("(ic p) f -> p ic f", p=128))
            wo_sb = sb.tile([128, IC, D], BF16, tag="wo")
            nc.sync.dma_start(
                wo_sb[:], wo_d[:].rearrange("(ic p) f -> p ic f", p=128))
            gamma_sb = sb.tile([128, D], F32, tag="gamma")
            nc.sync.dma_start(gamma_sb[:], gamma_d[:].to_broadcast([128, D]))
            if not no_transpose:
                ident = sb.tile([128, 128], BF16, tag="ident")
                make_identity(nc, ident[:])

            sms_all = sb.tile([128, NT], F32, tag="sms")
            rstd_all = sb.tile([128, NT], F32, tag="rstd")
            rstd_bf = sb.tile([128, NT], BF16, tag="rstdbf")
            xnT = [sb.tile([128, TOK], BF16, tag=f"xnT{ic}", name=f"xnT{ic}")
                   for ic in range(IC)]
            QT = sb.tile([128, TOK], BF16, tag="qt")
            KTt = sb.tile([128, TOK], BF16, tag="kt")
            # V (token-major) + 64 ones-columns per head for the Z row trick
            v_sb = [sb.tile([128, HPC, 128], BF16, tag=f"v{t}", name=f"v{t}")
                    for t in range(NT)]
            for t in range(NT):
                nc.vector.memset(v_sb[t][:, :, DH:128], 1.0)

            # ---- phase A: mean-square + rstd (token-major bf16 x) ----
            for tt in range(NT):
                x_t = sb.tile([128, D], BF16, tag="x", bufs=3)
                nc.sync.dma_start(x_t[:], xb_d[tt * 128:(tt + 1) * 128, :])
                sq_t = sb.tile([128, D], BF16, tag="sq", bufs=2)
                ssq = sb.tile([128, 1], F32, tag="ssq", bufs=2)
                # NB: vector.tensor_tensor_reduce fails at runtime on this
                # stack (INTERNAL error on NEFF execution) -- use ACT square
                nc.scalar.activation(sq_t[:], x_t[:], AF.Square,
                                     accum_out=ssq[:])
                nc.scalar.activation(sms_all[:, tt:tt + 1], ssq[:], AF.Sqrt,
                                     scale=1.0 / D)
            nc.vector.reciprocal(rstd_all[:], sms_all[:])
            nc.vector.tensor_copy(rstd_bf[:], rstd_all[:])
            nc.sync.dma_start(rs_dram[:], rstd_bf[:])
            # rstd replicated across partitions, token-major [128, TOK]
            rstdB = sb.tile([128, TOK], BF16, tag="rstdB")
            if no_bcast:
                nc.vector.memset(rstdB[:], 1.0)
            else:
                for t in range(NT):
                    nc.sync.dma_start(
                        rstdB[:, t * 128:(t + 1) * 128],
                        rs_dram[:, t:t + 1].rearrange(
                            "q o -> o q").to_broadcast([128, 128]))

            # ---- phase A2: xnT = xT * rstd (feature-major) ----
            for ic in range(IC):
                xt_t = sb.tile([128, TOK], BF16, tag="xt", bufs=2)
                nc.sync.dma_start(xt_t[:], xt_d[ic * 128:(ic + 1) * 128, :])
                nc.vector.tensor_mul(xnT[ic][:], xt_t[:], rstdB[:])

            # ---- phase B: projections (feature-major QT/KT/VT) ----
            for w_sb, dst in ((wq_sb, QT), (wk_sb, KTt)):
                for tg in range(TG):
                    pq = ps.tile([128, 512], F32, tag="pqk", bufs=2)
                    for ic in range(IC):
                        nc.tensor.matmul(
                            pq[:], w_sb[:, ic, :],
                            xnT[ic][:, tg * 512:(tg + 1) * 512],
                            start=(ic == 0), stop=(ic == IC - 1))
                    nc.scalar.copy(dst[:, tg * 512:(tg + 1) * 512], pq[:])
            # V: compute feature-major VT, then PE-transpose to token-major
            if no_transpose:
                for tt in range(NT):
                    pv = ps.tile([128, FPC], F32, tag="ptr", bufs=2)
                    for ic in range(IC):
                        nc.tensor.matmul(
                            pv[:], xnT[ic][:, tt * 128:(tt + 1) * 128],
                            wv_sb[:, ic, :], start=(ic == 0),
                            stop=(ic == IC - 1))
                    nc.vector.tensor_copy(
                        v_sb[tt][:, :, 0:DH],
                        pv[:].rearrange("p (h f) -> p h f", h=HPC))
            else:
              for tg in range(TG):
                pq = ps.tile([128, 512], F32, tag="pqk", bufs=2)
                for ic in range(IC):
                    nc.tensor.matmul(
                        pq[:], wv_sb[:, ic, :],
                        xnT[ic][:, tg * 512:(tg + 1) * 512],
                        start=(ic == 0), stop=(ic == IC - 1))
                vt_t = sb.tile([128, 512], BF16, tag="vt", bufs=2)
                nc.scalar.copy(vt_t[:], pq[:])
                for j in range(4):
                    tt = tg * 4 + j
                    ptr = ps.tile([128, 128], BF16, tag="ptr", bufs=2)
                    nc.tensor.transpose(
                        ptr[:], vt_t[:, j * 128:(j + 1) * 128], ident[:])
                    nc.vector.tensor_copy(
                        v_sb[tt][:, :, 0:DH],
                        ptr[:].rearrange("p (h f) -> p h f", h=HPC))

            # stop_after: 'proj' skips attention+a2a+out, 'attn' skips
            # a2a+out, 'a2a' skips out -- dummy out writes keep NEFF valid
            do_attn = stop_after not in ('xnt', 'proj')
            do_a2a = do_attn and stop_after != 'attn'
            do_out = do_a2a and stop_after != 'a2a'
            # ---- phase C: attention (transposed scores, fused Z) ----
            for b in range(B if do_attn else 0):
                for qq in range(QQ):
                    q0 = b * S + qq * QCH
                    dst = q0 // TPC
                    pa = [ps.tile([128, QCH], F32, tag=f"pa{h}", bufs=1,
                                  name=f"pa{h}_{b}_{qq}")
                          for h in range(HPC)]
                    for kt in range(KT):
                        gt = b * KT + kt
                        k0 = b * S + kt * 128
                        p_s = ps.tile([128, HPC * QCH], F32, tag="ps", bufs=1)
                        for h in range(HPC):
                            lo = h * DH
                            nc.tensor.matmul(
                                p_s[:, h * QCH:(h + 1) * QCH],
                                KTt[lo:lo + DH, k0:k0 + 128],
                                QT[lo:lo + DH, q0:q0 + QCH],
                                start=True, stop=True)
                        e_t = sb.tile([128, HPC * QCH], BF16, tag="e", bufs=3)
                        if no_merged_exp:
                            for h in range(HPC):
                                nc.scalar.activation(
                                    e_t[:, h * QCH:(h + 1) * QCH],
                                    p_s[:, h * QCH:(h + 1) * QCH],
                                    AF.Exp, scale=0.125)
                        else:
                            nc.scalar.activation(e_t[:], p_s[:], AF.Exp,
                                                 scale=0.125)
                        for h in range(HPC):
                            nc.tensor.matmul(
                                pa[h][:], v_sb[gt][:, h, :],
                                e_t[:, h * QCH:(h + 1) * QCH],
                                start=(kt == 0), stop=(kt == KT - 1))
                    for h in range(HPC):
                        rz = sb.tile([64, QCH], F32, tag="rz", bufs=2)
                        nc.vector.reciprocal(rz[:], pa[h][64:128, :])
                        an = sb.tile([64, QCH], BF16, tag="an", bufs=2)
                        nc.vector.tensor_mul(an[:], pa[h][0:64, :], rz[:])
                        nc.sync.dma_start(
                            bounce_in[dst, h * DH:(h + 1) * DH, :], an[:])

            # ---- phase D: all-to-all (head-shard -> token-shard) ----
            if do_a2a:
              nc.gpsimd.collective_compute(
                "AllToAll", mybir.AluOpType.bypass,
                replica_groups=[list(range(NCORES))],
                ins=[bounce_in[:].opt()],
                outs=[bounce_out[:].opt()])

            if not do_out:
                for lt in range(LT):
                    t0 = lt * 128
                    dtile = sb.tile([128, D], F32, tag="xr", bufs=2)
                    nc.sync.dma_start(dtile[:], xres_d[t0:t0 + 128, :])
                    nc.sync.dma_start(out_d[t0:t0 + 128, :], dtile[:])
            # ---- phase E: output projection + residual, token-sharded ----
            for lt in range(LT if do_out else 0):
                t0 = lt * 128
                at = sb.tile([128, NCORES, 128], BF16, tag="at", bufs=2)
                nc.sync.dma_start(
                    at[:],
                    bounce_out[:, :, t0:t0 + 128].rearrange("s f t -> f s t"))
                po = [ps.tile([128, 512], F32, tag="pqk", bufs=2,
                              name=f"po{lt}_{ng}")
                      for ng in range(2)]
                for ng in range(2):
                    for ic in range(IC):
                        nc.tensor.matmul(
                            po[ng][:], at[:, ic, :],
                            wo_sb[:, ic, ng * 512:(ng + 1) * 512],
                            start=(ic == 0), stop=(ic == IC - 1))
                x_r = sb.tile([128, D], F32, tag="xr", bufs=2)
                nc.sync.dma_start(x_r[:], xres_d[t0:t0 + 128, :])
                sq_r = sb.tile([128, D], BF16, tag="sq", bufs=2)
                ssq_r = sb.tile([128, 1], F32, tag="ssq", bufs=2)
                nc.scalar.activation(sq_r[:], x_r[:], AF.Square,
                                     accum_out=ssq_r[:])
                sms_r = sb.tile([128, 1], F32, tag="smsr", bufs=2)
                nc.scalar.activation(sms_r[:], ssq_r[:], AF.Sqrt, scale=1.0 / D)
                rstd_r = sb.tile([128, 1], F32, tag="rstdr", bufs=2)
                nc.vector.reciprocal(rstd_r[:], sms_r[:])
                xn_r = sb.tile([128, D], F32, tag="xnr", bufs=2)
                nc.vector.tensor_scalar_mul(xn_r[:], x_r[:], rstd_r[:])
                xg_r = sb.tile([128, D], F32, tag="xgr", bufs=2)
                nc.vector.tensor_mul(xg_r[:], xn_r[:], gamma_sb[:])
                ot = sb.tile([128, D], F32, tag="ot", bufs=2)
                for ng in range(2):
                    nc.vector.tensor_add(
                        ot[:, ng * 512:(ng + 1) * 512],
                        xg_r[:, ng * 512:(ng + 1) * 512], po[ng][:])
                nc.sync.dma_start(out_d[t0:t0 + 128, :], ot[:])

    nc.compile()
    return nc


_CACHE = {}


def _get_nc(B=2, S=2048):
    key = (B, S)
    if key not in _CACHE:
        _CACHE[key] = build(B, S)
    return _CACHE[key]


def make_in_maps(x, Wq, Wk, Wv, Wo, gamma, B, S):
    TOK = B * S
    TPC = TOK // NCORES
    bf = ml_dtypes.bfloat16
    x2d = np.ascontiguousarray(np.asarray(x, np.float32).reshape(TOK, D))
    xb = x2d.astype(bf)
    xt = np.ascontiguousarray(x2d.T.astype(bf))
    gam = np.asarray(gamma, np.float32).reshape(D)
    woT = np.ascontiguousarray(np.asarray(Wo, np.float32).T.astype(bf))
    gamma_in = np.ascontiguousarray(gam.reshape(1, D))
    in_maps = []
    for c in range(NCORES):
        fs = slice(c * FPC, (c + 1) * FPC)
        m = {
            "xb": xb,
            "xt": xt,
            "xres": np.ascontiguousarray(x2d[c * TPC:(c + 1) * TPC]),
            "wo": woT,
            "gamma": gamma_in,
        }
        for name, W in (("wq", Wq), ("wk", Wk), ("wv", Wv)):
            Wc = np.asarray(W, np.float32)[fs, :] * gam[None, :]
            m[name] = np.ascontiguousarray(Wc.T.astype(bf))
        in_maps.append(m)
    return in_maps


def kernel(x, attn_mask, Wq, Wk, Wv, Wo, gamma, _trace=False):
    B, S, _ = np.asarray(x).shape
    nc = _get_nc(B, S)
    in_maps = make_in_maps(x, Wq, Wk, Wv, Wo, gamma, B, S)
    res = run_bass_kernel_spmd(nc, in_maps, core_ids=list(range(NCORES)),
                               trace=_trace)
    out = np.concatenate([res.results[c]["out"] for c in range(NCORES)], axis=0)
    out = out.reshape(B, S, D).astype(np.float32)
    if _trace:
        kernel.last_results = res
    return out



# revision 11
# speedup vs baseline: 3.6496x; 3.6496x over previous
"""Trainium2 Bass kernel for MultiHeadAttention (RMSNorm + MHA + residual).

Reference computation (B=2, S=2048, D=1024, H=16):
    xn = x * rsqrt(mean(x^2, -1) + 1e-12) * gamma
    q/k/v = (xn @ W{q,k,v}.T) split into heads
    attn  = softmax(q k^T / sqrt(64)) v          (mask is zeros)
    out   = xn + (attn @ Wo.T)

Sharding: tensor-parallel over heads (2 heads/core on 8 cores) for
QKV/scores/softmax/attn-V, then an AllToAll switches to token sharding
for the output projection + residual. Token ownership is STRIPED:
core c owns tokens {g*512 + c*64 + r} so that block g's attention
output provides a 64-token stripe of EVERY core's share -- the AllToAll
is split into 8 chunked collectives (one per q-block) that overlap the
remaining attention compute, and the output projection overlaps too.

Key differences vs the v1 baseline:
  * rstd is computed from feature-major x only: sum-of-squares via an
    all-ones stationary matmul (replicates ssq across all psum
    partitions -- no DRAM broadcast round-trip, which cost ~400us),
    then 1/x via the fast DVE reciprocal and sqrt on ACT.
  * Q/K/V are projected from RAW x and scaled by rstd afterwards
    (linearity), removing the xn materialization entirely.
  * Scores psum (2 banks) and attnV accumulators are arranged so the
    scalar-engine exp (the critical resource: 128 x ~1.15us) overlaps
    matmuls via double-buffered score psum.
  * Z normalization uses reciprocal_approx_fast (~5x faster than DVE
    reciprocal; 18 bits is plenty for a bf16 result).
  * gamma is folded host-side into Wq/Wk/Wv and into the residual rows.
  * per-token rstd for the residual path rides along the A2A payload
    as a 129th feature row.
"""

import numpy as np
import ml_dtypes

import concourse.bacc as bacc
import concourse.mybir as mybir
import concourse.tile as tile
from concourse.bass_utils import run_bass_kernel_spmd
from concourse.masks import make_identity

F32 = mybir.dt.float32
BF16 = mybir.dt.bfloat16
AF = mybir.ActivationFunctionType

NCORES = 8
D = 1024
H = 16
DH = 64            # head dim
HPC = H // NCORES  # heads per core
FPC = HPC * DH     # attn features per core


def build(B=2, S=2048, debug_dump=False):
    TOK = B * S
    IC = D // 128        # input-feature chunks
    TG = TOK // 512      # 512-token groups
    TPC = TOK // NCORES  # tokens per core
    LT = TPC // 128      # phase-E token tiles per core
    KT = S // 128        # key tiles per batch
    QCH = 512            # q-block size
    NBLK = TOK // QCH    # attention q-blocks (= A2A chunks)
    NT = TOK // 128
    assert TPC == 512 and NBLK == 8

    nc = bacc.Bacc("TRN2", target_bir_lowering=False, debug=False,
                   num_devices=NCORES)
    xt_d = nc.dram_tensor("xt", [D, TOK], BF16, kind="ExternalInput")
    xres_d = nc.dram_tensor("xres", [TPC, D], F32, kind="ExternalInput")
    wq_d = nc.dram_tensor("wq", [D, FPC], BF16, kind="ExternalInput")
    wk_d = nc.dram_tensor("wk", [D, FPC], BF16, kind="ExternalInput")
    wv_d = nc.dram_tensor("wv", [D, FPC], BF16, kind="ExternalInput")
    wo_d = nc.dram_tensor("wo", [D, D], BF16, kind="ExternalInput")
    out_d = nc.dram_tensor("out", [TPC, D], F32, kind="ExternalOutput")
    dbg_d = (nc.dram_tensor("dbg", [512, 1024], BF16, kind="ExternalOutput")
             if debug_dump else None)

    with tile.TileContext(nc) as tc:
        with (
            tc.tile_pool(name="sb", bufs=1) as sb,
            tc.tile_pool(name="dram", bufs=1, space="DRAM") as dpool,
        ):
            # per-chunk A2A bounce buffers (separate tiles keep the
            # dependency tracking per-chunk)
            bin_g = [dpool.tile([NCORES, FPC + 1, 64], BF16, name=f"bin{g}")
                     for g in range(NBLK)]
            bout_g = [dpool.tile([NCORES, FPC + 1, 64], BF16, name=f"bout{g}")
                      for g in range(NBLK)]
            # warmup collective: absorbs the ~80us first-collective setup
            # (ring/credit init) during phase A/B while CC cores are idle
            warm_in = dpool.tile([NCORES, 64], BF16, name="warmin")
            warm_out = dpool.tile([NCORES, 64], BF16, name="warmout")
            nc.gpsimd.collective_compute(
                "AllToAll", mybir.AluOpType.bypass,
                replica_groups=[list(range(NCORES))],
                ins=[warm_in[:].opt()], outs=[warm_out[:].opt()])

            # ---- persistent weights / constants ----
            wq_sb = sb.tile([128, IC, FPC], BF16, tag="wq")
            wk_sb = sb.tile([128, IC, FPC], BF16, tag="wk")
            wv_sb = sb.tile([128, IC, FPC], BF16, tag="wv")
            for w_sb, w_d in ((wq_sb, wq_d), (wk_sb, wk_d), (wv_sb, wv_d)):
                nc.sync.dma_start(
                    w_sb[:], w_d[:].rearrange("(ic p) f -> p ic f", p=128))
            wo_sb = sb.tile([128, IC, D], BF16, tag="wo")
            nc.sync.dma_start(
                wo_sb[:], wo_d[:].rearrange("(ic p) f -> p ic f", p=128))
            ident = sb.tile([128, 128], BF16, tag="ident")
            make_identity(nc, ident[:])
            ones_sb = sb.tile([128, 128], BF16, tag="ones")
            nc.vector.memset(ones_sb[:], 1.0)

            QT = sb.tile([128, TOK], BF16, tag="qt")
            KTt = sb.tile([128, TOK], BF16, tag="kt")
            rstdB = sb.tile([128, TOK], BF16, tag="rstdB")
            # V (token-major): cols 0..63 per head are ONES (Z lands on
            # psum partitions 0..63 where reciprocal_approx_fast can read
            # it -- the custom DVE op mishandles partition offsets), cols
            # 64..127 are V so attn lands on partitions 64..127.
            v_sb = [sb.tile([128, HPC, 128], BF16, tag=f"v{t}", name=f"v{t}")
                    for t in range(NT)]
            for t in range(NT):
                nc.vector.memset(v_sb[t][:, :, 0:DH], 1.0)

            # ---- phase A+B fused, per 512-token group: ssq via all-ones
            # stationary matmul (replicated across psum partitions), rstd,
            # then Q/K/V = (W @ x_raw) * rstd ----
            with tc.tile_pool(name="psAB", bufs=1, space="PSUM") as psB:
                for tg in range(TG):
                    sl = slice(tg * 512, (tg + 1) * 512)
                    xtg = sb.tile([128, IC, 512], BF16, tag="xtg", bufs=3)
                    nc.sync.dma_start(
                        xtg[:], xt_d[:, sl].rearrange("(ic p) t -> p ic t",
                                                      p=128))
                    pq = psB.tile([128, 512], F32, tag="pqk", bufs=3,
                                  name=f"pq{tg}")
                    pk = psB.tile([128, 512], F32, tag="pqk", bufs=3,
                                  name=f"pk{tg}")
                    pv = psB.tile([128, 512], F32, tag="pqk", bufs=3,
                                  name=f"pv{tg}")
                    for w_sb, ps_t in ((wq_sb, pq), (wk_sb, pk), (wv_sb, pv)):
                        for ic in range(IC):
                            nc.tensor.matmul(
                                ps_t[:], w_sb[:, ic, :], xtg[:, ic, :],
                                start=(ic == 0), stop=(ic == IC - 1))
                    xsq = sb.tile([128, IC, 512], BF16, tag="xsq", bufs=2)
                    nc.vector.tensor_mul(xsq[:], xtg[:], xtg[:])
                    ssq = psB.tile([128, 512], F32, tag="ssq", bufs=2,
                                   name=f"ssq{tg}")
                    for ic in range(IC):
                        nc.tensor.matmul(
                            ssq[:], ones_sb[:], xsq[:, ic, :],
                            start=(ic == 0), stop=(ic == IC - 1))
                    inv_t = sb.tile([128, 512], F32, tag="inv", bufs=2)
                    nc.vector.reciprocal_approx_fast(inv_t[:], ssq[:])
                    # rstd = sqrt(D / ssq)
                    nc.scalar.activation(rstdB[:, sl], inv_t[:], AF.Sqrt,
                                         scale=float(D))
                    nc.vector.tensor_mul(QT[:, sl], pq[:], rstdB[:, sl])
                    nc.vector.tensor_mul(KTt[:, sl], pk[:], rstdB[:, sl])
                    vt_t = sb.tile([128, 512], BF16, tag="vt", bufs=2)
                    nc.vector.tensor_mul(vt_t[:], pv[:], rstdB[:, sl])
                    for j in range(4):
                        tt = tg * 4 + j
                        ptr = psB.tile([128, 128], BF16, tag="ptr", bufs=2)
                        nc.tensor.transpose(
                            ptr[:], vt_t[:, j * 128:(j + 1) * 128], ident[:])
                        nc.vector.tensor_copy(
                            v_sb[tt][:, :, DH:128],
                            ptr[:].rearrange("p (h f) -> p h f", h=HPC))

            if debug_dump:
                nc.sync.dma_start(dbg_d[0:128, :], QT[:, 0:1024])
                nc.sync.dma_start(dbg_d[128:256, :], KTt[:, 0:1024])

            # ---- phase C+E pool: scores(4) + attnV(2) + outproj(2) banks --
            with tc.tile_pool(name="psC", bufs=1, space="PSUM") as psC:
                # ---- phase C: attention, one A2A chunk per q-block ----
                for g in range(NBLK):
                    b, qq = divmod(g, S // QCH)
                    q0 = b * S + qq * QCH
                    pa = [psC.tile([128, QCH], F32, tag=f"pa{h}", bufs=1,
                                   name=f"pa{h}_{g}")
                          for h in range(HPC)]
                    for kt in range(KT):
                        gt = b * KT + kt
                        k0 = b * S + kt * 128
                        p_s = psC.tile([128, HPC * QCH], F32, tag="ps",
                                       bufs=2, name=f"ps_{g}_{kt}")
                        for h in range(HPC):
                            lo = h * DH
                            nc.tensor.matmul(
                                p_s[:, h * QCH:(h + 1) * QCH],
                                KTt[lo:lo + DH, k0:k0 + 128],
                                QT[lo:lo + DH, q0:q0 + QCH],
                                start=True, stop=True)
                        e_t = sb.tile([128, HPC * QCH], BF16, tag="e", bufs=3)
                        nc.scalar.activation(e_t[:], p_s[:], AF.Exp,
                                             scale=0.125)
                        for h in range(HPC):
                            nc.tensor.matmul(
                                pa[h][:], v_sb[gt][:, h, :],
                                e_t[:, h * QCH:(h + 1) * QCH],
                                start=(kt == 0), stop=(kt == KT - 1))
                    # normalize by Z (psum rows 0..63, see v_sb layout)
                    # and scatter the 8 x 64-token stripes into the bounce
                    for h in range(HPC):
                        rz = sb.tile([64, QCH], F32, tag="rz", bufs=2)
                        nc.vector.reciprocal_approx_fast(
                            rz[:], pa[h][0:64, :])
                        an = sb.tile([64, QCH], BF16, tag="an", bufs=4)
                        nc.vector.tensor_mul(an[:], pa[h][64:128, :], rz[:])
                        if debug_dump and g == 0 and h == 0:
                            nc.sync.dma_start(dbg_d[256:320, 0:512], an[:])
                            rzb = sb.tile([64, QCH], BF16, tag="rzb")
                            nc.vector.tensor_copy(rzb[:], rz[:])
                            nc.sync.dma_start(dbg_d[320:384, 0:512], rzb[:])
                            pab = sb.tile([64, QCH], BF16, tag="pab")
                            nc.vector.tensor_copy(pab[:], pa[h][64:128, :])
                            nc.sync.dma_start(dbg_d[384:448, 0:512], pab[:])
                            zb = sb.tile([64, QCH], BF16, tag="zb")
                            nc.vector.tensor_copy(zb[:], pa[h][0:64, :])
                            nc.sync.dma_start(dbg_d[448:512, 0:512], zb[:])
                        nc.sync.dma_start(
                            bin_g[g][:, h * DH:(h + 1) * DH, :]
                            .rearrange("s f r -> f s r"),
                            an[:].rearrange("f (s r) -> f s r", s=NCORES))
                    # per-token rstd rides along as feature row 128
                    nc.sync.dma_start(
                        bin_g[g][:, FPC:FPC + 1, :]
                        .rearrange("s o r -> o s r"),
                        rstdB[0:1, g * 512:(g + 1) * 512]
                        .rearrange("o (s r) -> o s r", s=NCORES))
                    nc.gpsimd.collective_compute(
                        "AllToAll", mybir.AluOpType.bypass,
                        replica_groups=[list(range(NCORES))],
                        ins=[bin_g[g][:].opt()],
                        outs=[bout_g[g][:].opt()])

                # ---- phase E: output projection + residual (overlaps C
                # via the chunked A2A dependencies) ----
                for lt in range(LT):
                    t0 = lt * 128
                    at = sb.tile([128, NCORES, 128], BF16, tag="at", bufs=2)
                    rstdE = sb.tile([128, 1], BF16, tag="rse", bufs=2)
                    for half in range(2):
                        gg = 2 * lt + half
                        nc.sync.dma_start(
                            at[:, :, half * 64:(half + 1) * 64],
                            bout_g[gg][:, 0:FPC, :]
                            .rearrange("s f r -> f s r"))
                        nc.sync.dma_start(
                            rstdE[half * 64:(half + 1) * 64, :],
                            bout_g[gg][0:1, FPC:FPC + 1, :]
                            .rearrange("s o r -> r (s o)"))
                    rstdE_f = sb.tile([128, 1], F32, tag="rsef", bufs=2)
                    nc.vector.tensor_copy(rstdE_f[:], rstdE[:])
                    po = [psC.tile([128, 512], F32, tag=f"po{ng}", bufs=1,
                                   name=f"po{ng}_{lt}")
                          for ng in range(2)]
                    for ng in range(2):
                        for s in range(NCORES):
                            nc.tensor.matmul(
                                po[ng][:], at[:, s, :],
                                wo_sb[:, s, ng * 512:(ng + 1) * 512],
                                start=(s == 0), stop=(s == NCORES - 1))
                    x_r = sb.tile([128, D], F32, tag="xr", bufs=2)
                    nc.sync.dma_start(x_r[:], xres_d[t0:t0 + 128, :])
                    # xres already carries gamma; xn*gamma = xres * rstd
                    xg = sb.tile([128, D], F32, tag="xg", bufs=2)
                    nc.vector.tensor_scalar_mul(xg[:], x_r[:], rstdE_f[:])
                    ot = sb.tile([128, D], F32, tag="ot", bufs=2)
                    for ng in range(2):
                        nc.vector.tensor_add(
                            ot[:, ng * 512:(ng + 1) * 512],
                            xg[:, ng * 512:(ng + 1) * 512], po[ng][:])
                    nc.sync.dma_start(out_d[t0:t0 + 128, :], ot[:])

    nc.compile()
    return nc


_CACHE = {}


def _get_nc(B=2, S=2048):
    key = (B, S)
    if key not in _CACHE:
        _CACHE[key] = build(B, S)
    return _CACHE[key]


def make_in_maps(x, Wq, Wk, Wv, Wo, gamma, B, S):
    TOK = B * S
    bf = ml_dtypes.bfloat16
    x2d = np.ascontiguousarray(np.asarray(x, np.float32).reshape(TOK, D))
    xt = np.ascontiguousarray(x2d.T.astype(bf))
    gam = np.asarray(gamma, np.float32).reshape(D)
    woT = np.ascontiguousarray(np.asarray(Wo, np.float32).T.astype(bf))
    # residual rows carry gamma already, striped: core c owns tokens
    # {g*512 + c*64 + r}
    xg_res = (x2d * gam[None, :]).reshape(NCORES, NCORES, 64, D)
    in_maps = []
    for c in range(NCORES):
        fs = slice(c * FPC, (c + 1) * FPC)
        m = {
            "xt": xt,
            "xres": np.ascontiguousarray(
                xg_res[:, c].reshape(TOK // NCORES, D)),
            "wo": woT,
        }
        for name, W in (("wq", Wq), ("wk", Wk), ("wv", Wv)):
            Wc = np.asarray(W, np.float32)[fs, :] * gam[None, :]
            m[name] = np.ascontiguousarray(Wc.T.astype(bf))
        in_maps.append(m)
    return in_maps


def kernel(x, attn_mask, Wq, Wk, Wv, Wo, gamma, _trace=False):
    B, S, _ = np.asarray(x).shape
    nc = _get_nc(B, S)
    in_maps = make_in_maps(x, Wq, Wk, Wv, Wo, gamma, B, S)
    res = run_bass_kernel_spmd(nc, in_maps, core_ids=list(range(NCORES)),
                               trace=_trace)
    # core c's rows are (g, r) stripes: out[g*512 + c*64 + r] = res[c][g*64+r]
    allres = np.stack([res.results[c]["out"] for c in range(NCORES)], axis=0)
    out = allres.reshape(NCORES, NCORES, 64, D).transpose(1, 0, 2, 3)
    out = out.reshape(B, S, D).astype(np.float32)
    if _trace:
        kernel.last_results = res
    return out
